# revision 13
# baseline (speedup 1.0000x reference)
"""APNB (asymmetric pyramid non-local block) on 8 TRN2 NeuronCores.

Data-parallel: one batch sample per core. Per core (x: [512, 9216] fp16),
x streams to SBUF ONCE and stays resident; out streams back fp16 (host
upcasts). Total HBM traffic ~20 MB/core.

  Algebraic restructure 1 (pool/conv commute, both linear):
      psp_pool(conv1x1(x, W, b)) == W @ psp_pool(x) + b
  Algebraic restructure 2 (fold q conv into the keys; S=110 < O):
      attnT = k_pool.T @ (Wq x + bq) = kq.T @ x + t 1^T
      kq = M @ pooled + r,  M = Wq.T Wk, r = Wq.T bk   (host-folded consts)
      t  = pooled.T (Wk.T bq) + bk.bq
  Restructure 3: adaptive avg pools over 1/3/6/8 grids refine to a 24x24
  grid of 4x4-px block sums xh;  pooled = xh @ A,  A [576, 110].
  Restructure 4 (v2: transpose-free finalize): kq / vT / t never go
  through pooled at all -- per grid-stripe st (<=120 cells, channels on
  partitions):
      y_st = xh[:, st].T @ M      (4 mm, contraction over C partitions)
      w_st = xh[:, st].T @ Wv.T   (4 mm)
      u_st = xh[:, st].T @ wkq    (4 mm, N=1)
      kq  += y_st[:, cb].T @ A_st (4 mm, contraction over stripe cells)
      vT  += A_st.T @ w_st        (1 mm)
      t   += A_st.T @ u_st        (1 mm)
  Every step is a matmul whose operands are already in the right layout,
  so the whole reduction pipeline accumulates DURING the input stream;
  after the last byte only the last (small) stripe's chain remains
  (~4 us) before pass 2 can start.  Block sums per chunk are built by
  three engines in the DMA shadow (DVE windowed reduce kt0-1, PE
  identity-mm kt2-3, GpSimd pairwise row-adds), as in v1.
  Chunks are sized [512, 1536*4, 1024, 1024, 512] so each chunk
  completes exactly one stripe -> PE stays dense (HAM never throttles)
  and all chunk boundaries are 512-multiples (pass-2 sub-chunks never
  straddle a chunk).

  Pass 2 (per 512-col sub-chunk, software-pipelined depth 4):
    attnT(i) | exp/denom/recip/mul(i-1) | ident+v-mm(i-2) | copies+DMA(i-3)
  PE per chunk: 4 attnT + 1 denom + 4 v-mm + 2 identity-residual mm.
  Residual: c0/c1 fused into the DVE psum->sbuf add, c2/c3 via identity
  matmul + ACT copy. Out DMA on the sync HWDGE queue.

Softmax needs no max-subtraction: logits are in [-8, 8] for this problem
family (checked against the reference; exp stays finite in fp16).
"""

import numpy as np

import concourse.bass as bass
import concourse.bacc as bacc
import concourse.tile as tile
import concourse.mybir as mybir
from concourse.bass_utils import run_bass_kernel_spmd

AF = mybir.ActivationFunctionType
F16 = np.float16
AX = mybir.AxisListType

B = 8
C = 512          # input/value channels
O = 256          # q/k channels
H = 96
W = 96
N = H * W        # 9216
S = 110          # pooled length 1+9+36+64
PSP = (1, 3, 6, 8)
NCORES = 8
SUB = 512        # columns per pass-2 sub-chunk
NSUB = N // SUB  # 18
KT = C // 128    # 4 channel tiles
G = 576          # 24x24 block grid

# input DMA chunks (cols); all 512-multiples, one stripe ready per chunk
CHUNK_COLS = (512, 1536, 1536, 1536, 1536, 1024, 1024, 512)
NBIG = len(CHUNK_COLS)
CHUNK_OFF = tuple(int(v) for v in np.cumsum((0,) + CHUNK_COLS)[:-1])
# rows of the image complete after chunk ci; derived pair/quad counts
ROWS = [(CHUNK_OFF[ci] + CHUNK_COLS[ci]) // W for ci in range(NBIG)]
PAIRS = [r // 2 for r in ROWS]
QUADS = [p // 2 for p in PAIRS]
# grid stripes (hb-row ranges of the 24x24 grid); stripe st becomes
# complete exactly at chunk STRIPE_READY^-1[st]
STRIPE_HB = ((0, 4), (4, 8), (8, 12), (12, 16), (16, 20), (20, 22),
             (22, 24))
NSTRIPE = len(STRIPE_HB)
STRIPE_READY = {1: 0, 2: 1, 3: 2, 4: 3, 5: 4, 6: 5, 7: 6}
for ci, st in STRIPE_READY.items():
    assert QUADS[ci] >= STRIPE_HB[st][1], (ci, st)
# pass-2 sub-chunk -> (chunk, local col offset)
SUBMAP = []
for sub in range(NSUB):
    g0 = sub * SUB
    ci = max(i for i in range(NBIG) if CHUNK_OFF[i] <= g0)
    assert g0 + SUB <= CHUNK_OFF[ci] + CHUNK_COLS[ci]
    SUBMAP.append((ci, g0 - CHUNK_OFF[ci]))


def _build_agg_matrix() -> np.ndarray:
    # pooled[c, s] = sum_g A[g, s] * xh[c, g]; xh = sum(4x4 block)
    A = np.zeros((G, S), np.float32)
    col = 0
    for s in PSP:
        hbs = 24 // s
        npx = (96 // s) ** 2
        for i in range(s):
            for j in range(s):
                for hb in range(i * hbs, (i + 1) * hbs):
                    for wb in range(j * hbs, (j + 1) * hbs):
                        A[hb * 24 + wb, col] = 1.0 / npx
                col += 1
    assert col == S
    return A


def _stage(a: np.ndarray) -> np.ndarray:
    """[T*128, F] -> partition-major [128, T*F] (contiguous per partition)."""
    t = a.shape[0] // 128
    return np.ascontiguousarray(
        a.reshape(t, 128, a.shape[1]).transpose(1, 0, 2).reshape(128, -1))


def build_nc() -> bacc.Bacc:
    nc = bacc.Bacc("TRN2", target_bir_lowering=False, debug=False,
                   num_devices=NCORES)
    f16 = mybir.dt.float16
    f32 = mybir.dt.float32

    def din(name, shape, dt=f16):
        return nc.dram_tensor(name, shape, dt, kind="ExternalInput").ap()

    x_d = din("x_st", [128, KT * N])                    # chunk-major
    m_d = din("m_st", [128, KT * C])                    # (Wq.T Wk).T staged
    wv_d = din("wv_st", [128, KT * C])                  # Wv.T staged
    a_d = din("a_st", [128, NSTRIPE * S])               # A grid-stripes
    wkq_d = din("wkq_st", [128, KT])                    # Wk.T bq column
    r_d = din("r_st", [1, C])                           # Wq.T bk row
    bv_d = din("bv_st", [1, C])
    ones_d = din("ones_st", [128, 512])
    id_d = din("id_st", [128, 128])
    tb_d = din("tb_st", [110, 1], f32)
    out_d = nc.dram_tensor("out_st", [128, NSUB * KT * SUB], f16,
                           kind="ExternalOutput").ap()  # [p, sub, kt, nn]

    mv = m_d.rearrange("p (k m) -> p k m", k=KT)
    wvv = wv_d.rearrange("p (k m) -> p k m", k=KT)
    av = a_d.rearrange("p (g s) -> p g s", g=NSTRIPE)
    outv = out_d.rearrange("p (ci g nn) -> p ci g nn", ci=NSUB, g=KT)

    from contextlib import ExitStack
    with tile.TileContext(nc) as tc, ExitStack() as ctx:
        consts = ctx.enter_context(tc.tile_pool(name="consts", bufs=1))
        resid = ctx.enter_context(tc.tile_pool(name="resid", bufs=1))

        x_sb = resid.tile([128, KT * N], f16)  # resident input (chunked)

        def xchunk(ci):
            base = KT * CHUNK_OFF[ci]
            cols = CHUNK_COLS[ci]
            return x_sb[:, base:base + KT * cols].rearrange(
                "p (k c) -> p k c", k=KT)

        def xchunk_d(ci):
            base = KT * CHUNK_OFF[ci]
            cols = CHUNK_COLS[ci]
            return x_d[:, base:base + KT * cols]

        id_sb = consts.tile([128, 128], f16)
        a_sb = consts.tile([128, NSTRIPE, S], f16)
        m_sb = consts.tile([128, KT, C], f16)
        wv_sb = consts.tile([128, KT, C], f16)
        wkq_sb = consts.tile([128, KT], f16)
        r_sb = consts.tile([1, C], f16)
        bv_sb = consts.tile([1, C], f16)
        ones_sb = consts.tile([128, 512], f16)
        tb_sb = consts.tile([110, 1], f32)

        xw = consts.tile([128, KT, 96, 24], f16)     # W-sums (4-px groups)
        w3tmp = consts.tile([128, 512, 2], f16)      # kt3 W-sum scratch
        xh2 = consts.tile([128, KT, 48, 24], f16)    # after h-pair round
        xh_sb = consts.tile([128, KT, 24, 24], f16)  # 24x24 block sums
        kq_sb = consts.tile([128, KT, S], f16)
        u_sb = consts.tile([128, NSTRIPE], f16)
        t_sb = consts.tile([110, 1], f32)
        vT_sb = consts.tile([110, C], f16)

        # x0 first so compute starts ASAP; m/wv right after (needed by
        # stripe 0 ~17us in); the rest on the ACT HWDGE queue.
        nc.sync.dma_start(out=id_sb, in_=id_d)
        nc.sync.dma_start(out=a_sb, in_=av)
        nc.sync.dma_start(out=xchunk(0), in_=xchunk_d(0))
        nc.sync.dma_start(out=m_sb, in_=mv)
        nc.sync.dma_start(out=wv_sb, in_=wvv)
        for ci in range(1, NBIG):
            nc.sync.dma_start(out=xchunk(ci), in_=xchunk_d(ci))
        nc.scalar.dma_start(out=ones_sb, in_=ones_d)
        nc.scalar.dma_start(out=wkq_sb, in_=wkq_d)
        nc.scalar.dma_start(out=r_sb, in_=r_d)
        nc.scalar.dma_start(out=bv_sb, in_=bv_d)
        nc.scalar.dma_start(out=tb_sb, in_=tb_d)

        xwf = xw.rearrange("p k a b -> p k (a b)")
        xwp = xw.rearrange("p k (h two) wb -> p k h two wb", two=2)
        xh2q = xh2.rearrange("p k (q two) wb -> p k q two wb", two=2)
        xhf = xh_sb.rearrange("p k a b -> p k (a b)")

        # ------- pass 1: pooling + kq/vT/t accumulation in DMA shadow ----
        with tc.tile_pool(name="p1kq", bufs=1, space="PSUM") as pkq, \
             tc.tile_pool(name="p1vt", bufs=1, space="PSUM") as pvt, \
             tc.tile_pool(name="p1t", bufs=1, space="PSUM") as ptp, \
             tc.tile_pool(name="p1w", bufs=1, space="PSUM") as p1w, \
             tc.tile_pool(name="p1yw", bufs=1, space="PSUM") as pyw, \
             tc.tile_pool(name="p1ysb", bufs=2) as pysb:
            kq_ps = pkq.tile([128, KT, S], f32, tag="kq")
            vT_ps = pvt.tile([110, C], f32, tag="vt")
            t_ps = ptp.tile([110, 1], f32, tag="t")

            ysb_t = [None] * NSTRIPE

            def emit_stripe_yw(st):
                h0, h1 = STRIPE_HB[st]
                c0, gsz = 24 * h0, 24 * (h1 - h0)
                yw_ps = pyw.tile([128, 3, C], f32, tag="yw", name="yw_ps")
                # y/w/u live in their own psum banks; start=True at k==0
                # resets has_written for the reused bank each stripe
                for k in range(KT):
                    nc.tensor.matmul(
                        yw_ps[0:gsz, 0, :], xhf[:, k, c0:c0 + gsz],
                        m_sb[:, k, :], start=(k == 0),
                        stop=(k == KT - 1), skip_group_check=True)
                for k in range(KT):
                    nc.tensor.matmul(
                        yw_ps[0:gsz, 1, :], xhf[:, k, c0:c0 + gsz],
                        wv_sb[:, k, :], start=(k == 0),
                        stop=(k == KT - 1), skip_group_check=True)
                for k in range(KT):
                    nc.tensor.matmul(
                        yw_ps[0:gsz, 2, 0:1], xhf[:, k, c0:c0 + gsz],
                        wkq_sb[:, k:k + 1], start=(k == 0),
                        stop=(k == KT - 1), skip_group_check=True)
                yw_sb = pysb.tile([128, 2, C], f16, tag="ysb", name="yw_sb")
                # split copies: kq mms gate on the y half only. All on ACT
                # (DVE carries the reduces with no slack; ACT has ~2us/iter)
                nc.scalar.copy(yw_sb[0:gsz, 0, :], yw_ps[0:gsz, 0, :])
                nc.scalar.copy(yw_sb[0:gsz, 1, :], yw_ps[0:gsz, 1, :])
                nc.scalar.copy(u_sb[0:gsz, st:st + 1],
                               yw_ps[0:gsz, 2, 0:1])
                ysb_t[st] = yw_sb

            def emit_accum(st, first, last):
                h0, h1 = STRIPE_HB[st]
                gsz = 24 * (h1 - h0)
                yw_sb = ysb_t[st]
                for cb in range(KT):
                    nc.tensor.matmul(
                        kq_ps[:, cb, :],
                        yw_sb[0:gsz, 0, cb * 128:(cb + 1) * 128],
                        a_sb[0:gsz, st, :], start=(first and cb == 0),
                        stop=last, skip_group_check=True)
                if last:
                    nc.scalar.copy(kq_sb, kq_ps)
                nc.tensor.matmul(
                    vT_ps, a_sb[0:gsz, st, :], yw_sb[0:gsz, 1, :],
                    start=first, stop=last, skip_group_check=True)
                nc.tensor.matmul(
                    t_ps, a_sb[0:gsz, st, :], u_sb[0:gsz, st:st + 1],
                    start=first, stop=last, skip_group_check=True)
                if first:
                    # bias rows fold in early so the tail stays short
                    for cb in range(KT):
                        nc.tensor.matmul(
                            kq_ps[:, cb, :],
                            r_sb[0:1, cb * 128:(cb + 1) * 128],
                            ones_sb[0:1, 0:S], start=False, stop=False,
                            skip_group_check=True)
                    nc.tensor.matmul(
                        vT_ps, ones_sb[0:1, 0:110], bv_sb,
                        start=False, stop=False, skip_group_check=True)
                if last:
                    nc.vector.tensor_scalar_add(t_sb, t_ps, tb_sb)
                    nc.vector.tensor_copy(vT_sb, vT_ps)

            # software-pipelined: stripe(st) is emitted one chunk AFTER its
            # data completes, so no engine queue ever holds an op whose
            # deps lie >1 chunk behind -- every FIFO streams stall-free.
            pp = qq = 0
            for ci in range(NBIG):
                cols = CHUNK_COLS[ci]
                g0, ng = CHUNK_OFF[ci] // 4, cols // 4
                xcg = xchunk(ci).rearrange("p k (g j) -> p k g j", j=4)
                # W-sums kt 0-1 on DVE (windowed reduce)
                with nc.allow_low_precision("block sums in fp16"):
                    nc.vector.reduce_sum(xwf[:, 0:2, g0:g0 + ng],
                                         xcg[:, 0:2, :, :], axis=AX.X)
                # W-sums kt 2 on PE (identity mm) + ACT copy
                wps = p1w.tile([128, 512], f32, tag="w", name="wps")
                for j in range(4):
                    nc.tensor.matmul(wps[:, 0:ng], id_sb,
                                     xcg[:, 2, :, j],
                                     start=(j == 0), stop=(j == 3),
                                     skip_group_check=True)
                nc.scalar.copy(xwf[:, 2, g0:g0 + ng], wps[:, 0:ng])
                # W-sums kt 3 on GpSimd (two paired adds), then H-sums
                # (two pairwise row-add rounds)
                with nc.allow_low_precision("block sums in fp16"):
                    nc.gpsimd.tensor_add(w3tmp[:, 0:ng, :],
                                         xcg[:, 3, :, 0:2],
                                         xcg[:, 3, :, 2:4])
                    nc.gpsimd.tensor_add(xwf[:, 3, g0:g0 + ng],
                                         w3tmp[:, 0:ng, 0],
                                         w3tmp[:, 0:ng, 1])
                    if PAIRS[ci] > pp:
                        nc.gpsimd.tensor_add(
                            xh2[:, :, pp:PAIRS[ci], :],
                            xwp[:, :, pp:PAIRS[ci], 0, :],
                            xwp[:, :, pp:PAIRS[ci], 1, :])
                        pp = PAIRS[ci]
                    if QUADS[ci] > qq:
                        nc.gpsimd.tensor_add(
                            xh_sb[:, :, qq:QUADS[ci], :],
                            xh2q[:, :, qq:QUADS[ci], 0, :],
                            xh2q[:, :, qq:QUADS[ci], 1, :])
                        qq = QUADS[ci]
                if ci - 1 in STRIPE_READY:
                    emit_stripe_yw(STRIPE_READY[ci - 1])
                if ci - 2 in STRIPE_READY:
                    st = STRIPE_READY[ci - 2]
                    emit_accum(st, first=(st == 0), last=False)
            emit_accum(STRIPE_READY[NBIG - 2], first=False, last=False)
            emit_stripe_yw(STRIPE_READY[NBIG - 1])
            emit_accum(STRIPE_READY[NBIG - 1], first=False, last=True)

        # ------- pass 2: software-pipelined depth 4 ----------------------
        with tc.tile_pool(name="atp", bufs=3, space="PSUM") as atp, \
             tc.tile_pool(name="dbp", bufs=1, space="PSUM") as dbp, \
             tc.tile_pool(name="opp", bufs=2, space="PSUM") as opp, \
             tc.tile_pool(name="p2sb", bufs=4) as p2sb, \
             tc.tile_pool(name="outp", bufs=4) as outp:
            at_t = [None] * NSUB
            exp_t = [None] * NSUB
            db_t = [None] * NSUB
            rc_t = [None] * NSUB
            an_t = [None] * NSUB
            oa_t = [None] * NSUB
            ob_t = [None] * NSUB

            def xsub(c):
                ci, l0 = SUBMAP[c]
                return xchunk(ci)[:, :, l0:l0 + SUB]

            for i in range(NSUB + 3):
                a, e, o, f = i, i - 1, i - 2, i - 3
                if a < NSUB:               # attnT(a): PE x4
                    xs = xsub(a)
                    at = atp.tile([110, SUB], f32, tag="at")
                    for k in range(KT):
                        nc.tensor.matmul(
                            at, kq_sb[:, k, :], xs[:, k, :],
                            start=(k == 0), stop=(k == KT - 1))
                    at_t[a] = at
                if 0 <= e < NSUB:          # exp(e): ACT
                    ex = p2sb.tile([110, SUB], f16, tag="exp")
                    nc.scalar.activation(ex, at_t[e], AF.Exp,
                                         bias=t_sb, scale=1.0)
                    exp_t[e] = ex
                if 0 <= f:                 # copies + DMA out(f)
                    xs = xsub(f)
                    ot = outp.tile([128, KT, SUB], f16, tag="out")
                    with nc.allow_low_precision("fp16 out"):
                        nc.vector.tensor_add(ot[:, 0:2, :], oa_t[f],
                                             xs[:, 0:2, :])
                    nc.scalar.copy(ot[:, 2:4, :], ob_t[f])
                    nc.sync.dma_start(out=outv[:, f, :, :], in_=ot)
                if 0 <= o < NSUB:          # ident-residual(o): PE x2
                    xs = xsub(o)
                    ob = opp.tile([128, 2, SUB], f32, tag="o")
                    for j, cc in enumerate((2, 3)):
                        nc.tensor.matmul(ob[:, j, :], id_sb,
                                         xs[:, cc, :],
                                         start=True, stop=False,
                                         skip_group_check=True)
                    ob_t[o] = ob
                if 0 <= e < NSUB:          # denom(e): PE x1
                    db = dbp.tile([110, SUB], f32, tag="db")
                    nc.tensor.matmul(db, ones_sb[0:110, 0:110],
                                     exp_t[e], start=True, stop=True)
                    db_t[e] = db
                    rc = p2sb.tile([110, SUB], f32, tag="recip")
                    nc.vector.reciprocal_approx_fast(rc, db_t[e])
                    rc_t[e] = rc
                if 0 <= o < NSUB:          # v-mm (o): PE x4
                    for j, cc in enumerate((2, 3)):
                        nc.tensor.matmul(
                            ob_t[o][:, j, :],
                            vT_sb[:, cc * 128:(cc + 1) * 128],
                            an_t[o], start=False, stop=True,
                            skip_group_check=True)
                    oa = opp.tile([128, 2, SUB], f32, tag="o")
                    for j, cc in enumerate((0, 1)):
                        nc.tensor.matmul(
                            oa[:, j, :],
                            vT_sb[:, cc * 128:(cc + 1) * 128],
                            an_t[o], start=True, stop=True,
                            skip_group_check=True)
                    oa_t[o] = oa
                if 0 <= e < NSUB:          # attn weights (e): GpSimd
                    an = p2sb.tile([110, SUB], f16, tag="attn")
                    with nc.allow_low_precision("softmax in fp16"):
                        nc.gpsimd.tensor_mul(an, exp_t[e], rc_t[e])
                    an_t[e] = an

    nc.compile()
    return nc


_NC_CACHE = None


def _get_nc() -> bacc.Bacc:
    global _NC_CACHE
    if _NC_CACHE is None:
        _NC_CACHE = build_nc()
    return _NC_CACHE


def _prep_in_maps(x, Wq, bq, Wk, bk, Wv, bv):
    A = _build_agg_matrix()
    a_st = np.zeros((128, NSTRIPE, S), np.float32)
    for st, (h0, h1) in enumerate(STRIPE_HB):
        c0, gsz = 24 * h0, 24 * (h1 - h0)
        a_st[:gsz, st, :] = A[c0:c0 + gsz, :]
    Wq64 = Wq.astype(np.float64)
    Wk64 = Wk.astype(np.float64)
    shared = {
        "a_st": np.ascontiguousarray(a_st.reshape(128, -1)).astype(F16),
        "m_st": _stage((Wk64.T @ Wq64).astype(np.float32)).astype(F16),
        "wv_st": _stage(np.ascontiguousarray(Wv.T)).astype(F16),
        "wkq_st": np.ascontiguousarray(
            (Wk64.T @ bq).astype(np.float32).reshape(KT, 128).T).astype(F16),
        "r_st": (Wq64.T @ bk).astype(np.float32).reshape(1, C).astype(F16),
        "bv_st": np.ascontiguousarray(bv.reshape(1, C)).astype(F16),
        "ones_st": np.ones((128, 512), dtype=F16),
        "id_st": np.eye(128, dtype=np.float32).astype(F16),
        "tb_st": np.full((110, 1), float(bk @ bq), dtype=np.float32),
    }
    in_maps = []
    for i in range(NCORES):
        xi_f16 = np.ascontiguousarray(x[i].reshape(C, N)).astype(F16)
        # x: chunk-major [p, (ci: kt, cols)]
        parts = []
        for ci in range(NBIG):
            o, cols = CHUNK_OFF[ci], CHUNK_COLS[ci]
            blk = xi_f16[:, o:o + cols].reshape(KT, 128, cols)
            parts.append(blk.transpose(1, 0, 2).reshape(128, -1))
        x_st = np.ascontiguousarray(np.concatenate(parts, axis=1))
        m = dict(shared)
        m["x_st"] = x_st
        in_maps.append(m)
    return in_maps


def _unstage_out(o: np.ndarray) -> np.ndarray:
    # [128, NSUB*KT*SUB] fp16 -> [C, H, W] fp32
    return np.ascontiguousarray(
        o.reshape(128, NSUB, KT, SUB).transpose(2, 0, 1, 3)
        .reshape(C, N)).astype(np.float32).reshape(C, H, W)


def _install_ntff_hook():
    """The agent image ships no antenv.axon_hooks module, so trace=True
    under axon crashes on import. Recreate the tiny hook-holder module and
    register trn_boot's ctypes NTFF hook so neuron-profile timing works."""
    import sys
    import types
    if "antenv.axon_hooks" in sys.modules:
        return
    mod = types.ModuleType("antenv.axon_hooks")
    holder = {"h": None}
    mod.set_axon_ntff_profile_hook = lambda h: holder.__setitem__("h", h)
    mod.get_axon_ntff_profile_hook = lambda: holder["h"]
    sys.modules["antenv.axon_hooks"] = mod
    try:
        import antenv
        antenv.axon_hooks = mod
    except ImportError:
        pass
    try:
        from trn_agent_boot.trn_boot import _ntff_profile_via_ctypes
        mod.set_axon_ntff_profile_hook(
            _ntff_profile_via_ctypes("/opt/axon/libaxon_pjrt.so"))
    except Exception as e:  # degrade to no profiling
        print(f"ntff hook install failed: {e}")


def _run(trace: bool, **inputs):
    if trace:
        _install_ntff_hook()
        import concourse.bass_utils as bu
        bu.upload_artifacts = lambda tmpdir: tmpdir  # no cloud bucket here
    nc = _get_nc()
    in_maps = _prep_in_maps(
        inputs["x"], inputs["Wq"], inputs["bq"], inputs["Wk"], inputs["bk"],
        inputs["Wv"], inputs["bv"])
    res = run_bass_kernel_spmd(nc, in_maps, core_ids=list(range(NCORES)),
                               trace=trace)
    out = np.stack([
        _unstage_out(np.asarray(res.results[i]["out_st"]))
        for i in range(NCORES)
    ]).astype(np.float32)
    return out, res


def kernel(**inputs) -> np.ndarray:
    out, _ = _run(False, **inputs)
    return out


def kernel_profiled(**inputs):
    out, res = _run(True, **inputs)
    return out, res


# revision 20
# speedup vs baseline: 1.0059x; 1.0059x over previous
"""APNB (asymmetric pyramid non-local block) on 8 TRN2 NeuronCores.

Data-parallel: one batch sample per core. Per core (x: [512, 9216] fp16),
x streams to SBUF ONCE and stays resident; out streams back fp16 (host
upcasts). Total HBM traffic ~20 MB/core.

  Algebraic restructure 1 (pool/conv commute, both linear):
      psp_pool(conv1x1(x, W, b)) == W @ psp_pool(x) + b
  Algebraic restructure 2 (fold q conv into the keys; S=110 < O):
      attnT = k_pool.T @ (Wq x + bq) = kq.T @ x + t 1^T
      kq = M @ pooled + r,  M = Wq.T Wk, r = Wq.T bk   (host-folded consts)
      t  = pooled.T (Wk.T bq) + bk.bq
  Restructure 3: adaptive avg pools over 1/3/6/8 grids refine to a 24x24
  grid of 4x4-px block sums xh;  pooled = xh @ A,  A [576, 110].
  Restructure 4 (v2: transpose-free finalize): kq / vT / t never go
  through pooled at all -- per grid-stripe st (<=120 cells, channels on
  partitions):
      y_st = xh[:, st].T @ M      (4 mm, contraction over C partitions)
      w_st = xh[:, st].T @ Wv.T   (4 mm)
      u_st = xh[:, st].T @ wkq    (4 mm, N=1)
      kq  += y_st[:, cb].T @ A_st (4 mm, contraction over stripe cells)
      vT  += A_st.T @ w_st        (1 mm)
      t   += A_st.T @ u_st        (1 mm)
  Every step is a matmul whose operands are already in the right layout,
  so the whole reduction pipeline accumulates DURING the input stream;
  after the last byte only the last (small) stripe's chain remains
  (~4 us) before pass 2 can start.  Block sums per chunk are built by
  three engines in the DMA shadow (DVE windowed reduce kt0-1, PE
  identity-mm kt2-3, GpSimd pairwise row-adds), as in v1.
  Chunks are sized [512, 1536*4, 1024, 1024, 512] so each chunk
  completes exactly one stripe -> PE stays dense (HAM never throttles)
  and all chunk boundaries are 512-multiples (pass-2 sub-chunks never
  straddle a chunk).

  Pass 2 (per 512-col sub-chunk, software-pipelined depth 4):
    attnT(i) | exp/denom/recip/mul(i-1) | ident+v-mm(i-2) | copies+DMA(i-3)
  PE per chunk: 4 attnT + 1 denom + 4 v-mm + 2 identity-residual mm.
  Residual: c0/c1 fused into the DVE psum->sbuf add, c2/c3 via identity
  matmul + ACT copy. Out DMA on the sync HWDGE queue.

Softmax needs no max-subtraction: logits are in [-8, 8] for this problem
family (checked against the reference; exp stays finite in fp16).
"""

import numpy as np

import concourse.bass as bass
import concourse.bacc as bacc
import concourse.tile as tile
import concourse.mybir as mybir
from concourse.bass_utils import run_bass_kernel_spmd

AF = mybir.ActivationFunctionType
F16 = np.float16
AX = mybir.AxisListType

B = 8
C = 512          # input/value channels
O = 256          # q/k channels
H = 96
W = 96
N = H * W        # 9216
S = 110          # pooled length 1+9+36+64
PSP = (1, 3, 6, 8)
NCORES = 8
SUB = 512        # columns per pass-2 sub-chunk
NSUB = N // SUB  # 18
KT = C // 128    # 4 channel tiles
G = 576          # 24x24 block grid

# input DMA chunks (cols); 1536 = 16 rows = 4 hb-rows each, so every
# chunk is both 512-aligned (pass-2 subs) and hb-row-aligned (direct
# block-sum matmuls)
CHUNK_COLS = (1536,) * 6
NBIG = len(CHUNK_COLS)
CHUNK_OFF = tuple(int(v) for v in np.cumsum((0,) + CHUNK_COLS)[:-1])
# rows of the image complete after chunk ci; derived pair/quad counts
ROWS = [(CHUNK_OFF[ci] + CHUNK_COLS[ci]) // W for ci in range(NBIG)]
PAIRS = [r // 2 for r in ROWS]
QUADS = [p // 2 for p in PAIRS]
# grid stripes (hb-row ranges of the 24x24 grid); stripe st is complete
# once chunk STRIPE_READY[st] has been pooled (st5 AND st6 at chunk 5)
STRIPE_HB = ((0, 4), (4, 8), (8, 12), (12, 16), (16, 20), (20, 22),
             (22, 24))
NSTRIPE = len(STRIPE_HB)
STRIPE_READY = {0: 0, 1: 1, 2: 2, 3: 3, 4: 4, 5: 5, 6: 5}
for st, ci in STRIPE_READY.items():
    assert QUADS[ci] >= STRIPE_HB[st][1], (ci, st)
# pass-2 sub-chunk -> (chunk, local col offset)
SUBMAP = []
for sub in range(NSUB):
    g0 = sub * SUB
    ci = max(i for i in range(NBIG) if CHUNK_OFF[i] <= g0)
    assert g0 + SUB <= CHUNK_OFF[ci] + CHUNK_COLS[ci]
    SUBMAP.append((ci, g0 - CHUNK_OFF[ci]))


def _build_agg_matrix() -> np.ndarray:
    # pooled[c, s] = sum_g A[g, s] * xh[c, g]; xh = sum(4x4 block)
    A = np.zeros((G, S), np.float32)
    col = 0
    for s in PSP:
        hbs = 24 // s
        npx = (96 // s) ** 2
        for i in range(s):
            for j in range(s):
                for hb in range(i * hbs, (i + 1) * hbs):
                    for wb in range(j * hbs, (j + 1) * hbs):
                        A[hb * 24 + wb, col] = 1.0 / npx
                col += 1
    assert col == S
    return A


def _stage(a: np.ndarray) -> np.ndarray:
    """[T*128, F] -> partition-major [128, T*F] (contiguous per partition)."""
    t = a.shape[0] // 128
    return np.ascontiguousarray(
        a.reshape(t, 128, a.shape[1]).transpose(1, 0, 2).reshape(128, -1))


def build_nc() -> bacc.Bacc:
    nc = bacc.Bacc("TRN2", target_bir_lowering=False, debug=False,
                   num_devices=NCORES)
    f16 = mybir.dt.float16
    f32 = mybir.dt.float32

    def din(name, shape, dt=f16):
        return nc.dram_tensor(name, shape, dt, kind="ExternalInput").ap()

    x_d = din("x_st", [128, KT * N])                    # chunk-major
    m_d = din("m_st", [128, KT * C])                    # (Wq.T Wk).T staged
    wv_d = din("wv_st", [128, KT * C])                  # Wv.T staged
    a_d = din("a_st", [128, NSTRIPE * S])               # A grid-stripes
    z_d = din("z_st", [128, KT])                        # t = kq.T z + tb
    r_d = din("r_st", [1, C])                           # Wq.T bk row
    bv_d = din("bv_st", [1, C])
    ones_d = din("ones_st", [128, 512])
    id_d = din("id_st", [128, 128])
    tb_d = din("tb_st", [110, 1], f32)
    out_d = nc.dram_tensor("out_st", [128, NSUB * KT * SUB], f16,
                           kind="ExternalOutput").ap()  # [p, sub, kt, nn]

    mv = m_d.rearrange("p (k m) -> p k m", k=KT)
    wvv = wv_d.rearrange("p (k m) -> p k m", k=KT)
    av = a_d.rearrange("p (g s) -> p g s", g=NSTRIPE)
    outv = out_d.rearrange("p (ci g nn) -> p ci g nn", ci=NSUB, g=KT)

    from contextlib import ExitStack
    with tile.TileContext(nc) as tc, ExitStack() as ctx:
        consts = ctx.enter_context(tc.tile_pool(name="consts", bufs=1))
        resid = ctx.enter_context(tc.tile_pool(name="resid", bufs=1))

        x_sb = resid.tile([128, KT * N], f16)  # resident input (chunked)

        def xchunk(ci):
            base = KT * CHUNK_OFF[ci]
            cols = CHUNK_COLS[ci]
            return x_sb[:, base:base + KT * cols].rearrange(
                "p (k c) -> p k c", k=KT)

        def xchunk_d(ci):
            base = KT * CHUNK_OFF[ci]
            cols = CHUNK_COLS[ci]
            return x_d[:, base:base + KT * cols]

        id_sb = consts.tile([128, 128], f16)
        a_sb = consts.tile([128, NSTRIPE, S], f16)
        m_sb = consts.tile([128, KT, C], f16)
        wv_sb = consts.tile([128, KT, C], f16)
        z_sb = consts.tile([128, KT], f16)
        r_sb = consts.tile([1, C], f16)
        bv_sb = consts.tile([1, C], f16)
        ones_sb = consts.tile([128, 512], f16)
        tb_sb = consts.tile([110, 1], f32)

        xw = consts.tile([128, 2, 96, 24], f16)      # kt0-1 W-sums
        xh2 = consts.tile([128, 2, 48, 24], f16)     # after h-pair round
        xh_sb = consts.tile([128, KT, 24, 24], f16)  # 24x24 block sums
        kq_sb = consts.tile([128, KT, S], f16)
        t_sb = consts.tile([110, 1], f32)
        vT_sb = consts.tile([110, C], f16)

        # x alone on the sync ring right after the tiny id/a so the
        # stream is never delayed; m/wv lead the ACT ring (needed by
        # stripe 0 ~17us in) and trickle in concurrently.
        nc.sync.dma_start(out=id_sb, in_=id_d)
        nc.sync.dma_start(out=a_sb, in_=av)
        for ci in range(NBIG):
            nc.sync.dma_start(out=xchunk(ci), in_=xchunk_d(ci))
        nc.scalar.dma_start(out=m_sb, in_=mv)
        nc.scalar.dma_start(out=wv_sb, in_=wvv)
        nc.scalar.dma_start(out=ones_sb, in_=ones_d)
        nc.scalar.dma_start(out=z_sb, in_=z_d)
        nc.scalar.dma_start(out=r_sb, in_=r_d)
        nc.scalar.dma_start(out=bv_sb, in_=bv_d)
        nc.scalar.dma_start(out=tb_sb, in_=tb_d)

        xwf = xw.rearrange("p k a b -> p k (a b)")
        xwp = xw.rearrange("p k (h two) wb -> p k h two wb", two=2)
        xh2q = xh2.rearrange("p k (q two) wb -> p k q two wb", two=2)
        xhf = xh_sb.rearrange("p k a b -> p k (a b)")

        # ------- pass 1: pooling + kq/vT accumulation in DMA shadow ------
        with tc.tile_pool(name="p1kq", bufs=1, space="PSUM") as pkq, \
             tc.tile_pool(name="p1vt", bufs=1, space="PSUM") as pvt, \
             tc.tile_pool(name="p1t", bufs=1, space="PSUM") as ptp, \
             tc.tile_pool(name="p1xh", bufs=2, space="PSUM") as pxh, \
             tc.tile_pool(name="p1yw", bufs=1, space="PSUM") as pyw, \
             tc.tile_pool(name="p1ysb", bufs=2) as pysb:
            kq_ps = pkq.tile([128, KT, S], f32, tag="kq")
            vT_ps = pvt.tile([110, C], f32, tag="vt")

            ysb_t = [None] * NSTRIPE

            def emit_stripe_yw(st):
                h0, h1 = STRIPE_HB[st]
                c0, gsz = 24 * h0, 24 * (h1 - h0)
                yw_ps = pyw.tile([128, 2, C], f32, tag="yw", name="yw_ps")
                # y/w live in their own psum banks; start=True at k==0
                # resets has_written for the reused bank each stripe
                for k in range(KT):
                    nc.tensor.matmul(
                        yw_ps[0:gsz, 0, :], xhf[:, k, c0:c0 + gsz],
                        m_sb[:, k, :], start=(k == 0),
                        stop=(k == KT - 1), skip_group_check=True)
                for k in range(KT):
                    nc.tensor.matmul(
                        yw_ps[0:gsz, 1, :], xhf[:, k, c0:c0 + gsz],
                        wv_sb[:, k, :], start=(k == 0),
                        stop=(k == KT - 1), skip_group_check=True)
                yw_sb = pysb.tile([128, 2, C], f16, tag="ysb", name="yw_sb")
                # split copies: kq mms gate on the y half only. All on ACT
                # (DVE carries the reduces with no slack; ACT has slack)
                nc.scalar.copy(yw_sb[0:gsz, 0, :], yw_ps[0:gsz, 0, :])
                nc.scalar.copy(yw_sb[0:gsz, 1, :], yw_ps[0:gsz, 1, :])
                ysb_t[st] = yw_sb

            def emit_accum(st, first, last):
                h0, h1 = STRIPE_HB[st]
                gsz = 24 * (h1 - h0)
                yw_sb = ysb_t[st]
                for cb in range(KT):
                    nc.tensor.matmul(
                        kq_ps[:, cb, :],
                        yw_sb[0:gsz, 0, cb * 128:(cb + 1) * 128],
                        a_sb[0:gsz, st, :], start=(first and cb == 0),
                        stop=last, skip_group_check=True)
                if last:
                    nc.scalar.copy(kq_sb, kq_ps)
                    # t = kq.T z + (bk.bq - r.z)  (z solves Wq z = bq)
                    t_ps = ptp.tile([110, 1], f32, tag="t")
                    for k in range(KT):
                        nc.tensor.matmul(
                            t_ps, kq_sb[:, k, :], z_sb[:, k:k + 1],
                            start=(k == 0), stop=(k == KT - 1),
                            skip_group_check=True)
                    nc.vector.tensor_scalar_add(t_sb, t_ps, tb_sb)
                nc.tensor.matmul(
                    vT_ps, a_sb[0:gsz, st, :], yw_sb[0:gsz, 1, :],
                    start=first, stop=last, skip_group_check=True)
                if first:
                    # bias rows fold in early so the tail stays short
                    for cb in range(KT):
                        nc.tensor.matmul(
                            kq_ps[:, cb, :],
                            r_sb[0:1, cb * 128:(cb + 1) * 128],
                            ones_sb[0:1, 0:S], start=False, stop=False,
                            skip_group_check=True)
                    nc.tensor.matmul(
                        vT_ps, ones_sb[0:1, 0:110], bv_sb,
                        start=False, stop=False, skip_group_check=True)
                if last:
                    nc.vector.tensor_copy(vT_sb, vT_ps)

            # software-pipelined: stripe(st) yw-mms are emitted one chunk
            # AFTER st's data completes and the kq/vT accumulation one
            # more chunk later, so no engine queue ever holds an op whose
            # deps lie <1 chunk back -- every FIFO streams stall-free.
            for ci in range(NBIG):
                cols = CHUNK_COLS[ci]
                g0, ng = CHUNK_OFF[ci] // 4, cols // 4
                hb0 = CHUNK_OFF[ci] // 384  # 4 hb-rows per chunk
                xcg = xchunk(ci).rearrange("p k (g j) -> p k g j", j=4)
                xcb = xchunk(ci).rearrange(
                    "p k (hb r wb j) -> p k hb r wb j", r=4, wb=24, j=4)
                # kt 0-1: W-sums on DVE (windowed reduce) ...
                with nc.allow_low_precision("block sums in fp16"):
                    nc.vector.reduce_sum(xwf[:, 0:2, g0:g0 + ng],
                                         xcg[:, 0:2, :, :], axis=AX.X)
                # ... then H-sums on GpSimd: two pairwise row-add rounds
                with nc.allow_low_precision("block sums in fp16"):
                    nc.gpsimd.tensor_add(
                        xh2[:, :, PAIRS[ci] - 8:PAIRS[ci], :],
                        xwp[:, :, PAIRS[ci] - 8:PAIRS[ci], 0, :],
                        xwp[:, :, PAIRS[ci] - 8:PAIRS[ci], 1, :])
                    nc.gpsimd.tensor_add(
                        xh_sb[:, 0:2, QUADS[ci] - 4:QUADS[ci], :],
                        xh2q[:, :, QUADS[ci] - 4:QUADS[ci], 0, :],
                        xh2q[:, :, QUADS[ci] - 4:QUADS[ci], 1, :])
                # kt 2-3: block sums DIRECT from x on PE: 16 identity mms
                # (one per in-block pixel offset) into one psum bank
                xhp = pxh.tile([128, 2, 4, 24], f32, tag="xh", name="xhp")
                for rr in range(4):
                    for j in range(4):
                        nc.tensor.matmul(
                            xhp, id_sb, xcb[:, 2:4, :, rr, :, j],
                            start=(rr == 0 and j == 0),
                            stop=(rr == 3 and j == 3),
                            skip_group_check=True)
                nc.scalar.copy(xh_sb[:, 2:4, hb0:hb0 + 4, :], xhp)
                if ci >= 1:
                    emit_stripe_yw(ci - 1)
                if ci >= 2:
                    emit_accum(ci - 2, first=(ci == 2), last=False)
            emit_stripe_yw(NBIG - 1)         # st5
            emit_accum(NBIG - 2, first=False, last=False)
            emit_stripe_yw(NBIG)             # st6
            emit_accum(NBIG - 1, first=False, last=False)
            emit_accum(NBIG, first=False, last=True)

        # ------- pass 2: software-pipelined depth 4 ----------------------
        with tc.tile_pool(name="atp", bufs=3, space="PSUM") as atp, \
             tc.tile_pool(name="dbp", bufs=1, space="PSUM") as dbp, \
             tc.tile_pool(name="opp", bufs=2, space="PSUM") as opp, \
             tc.tile_pool(name="p2sb", bufs=4) as p2sb, \
             tc.tile_pool(name="outp", bufs=4) as outp:
            at_t = [None] * NSUB
            exp_t = [None] * NSUB
            db_t = [None] * NSUB
            rc_t = [None] * NSUB
            an_t = [None] * NSUB
            oa_t = [None] * NSUB
            ob_t = [None] * NSUB

            def xsub(c):
                ci, l0 = SUBMAP[c]
                return xchunk(ci)[:, :, l0:l0 + SUB]

            for i in range(NSUB + 3):
                a, e, o, f = i, i - 1, i - 2, i - 3
                if a < NSUB:               # attnT(a): PE x4
                    xs = xsub(a)
                    at = atp.tile([110, SUB], f32, tag="at")
                    for k in range(KT):
                        nc.tensor.matmul(
                            at, kq_sb[:, k, :], xs[:, k, :],
                            start=(k == 0), stop=(k == KT - 1))
                    at_t[a] = at
                if 0 <= e < NSUB:          # exp(e): ACT
                    ex = p2sb.tile([110, SUB], f16, tag="exp")
                    nc.scalar.activation(ex, at_t[e], AF.Exp,
                                         bias=t_sb, scale=1.0)
                    exp_t[e] = ex
                if 0 <= f:                 # copies + DMA out(f)
                    xs = xsub(f)
                    ot = outp.tile([128, KT, SUB], f16, tag="out")
                    with nc.allow_low_precision("fp16 out"):
                        nc.vector.tensor_add(ot[:, 0:2, :], oa_t[f],
                                             xs[:, 0:2, :])
                    nc.scalar.copy(ot[:, 2:4, :], ob_t[f])
                    nc.sync.dma_start(out=outv[:, f, :, :], in_=ot)
                if 0 <= o < NSUB:          # ident-residual(o): PE x2
                    xs = xsub(o)
                    ob = opp.tile([128, 2, SUB], f32, tag="o")
                    for j, cc in enumerate((2, 3)):
                        nc.tensor.matmul(ob[:, j, :], id_sb,
                                         xs[:, cc, :],
                                         start=True, stop=False,
                                         skip_group_check=True)
                    ob_t[o] = ob
                if 0 <= e < NSUB:          # denom(e): PE x1
                    db = dbp.tile([110, SUB], f32, tag="db")
                    nc.tensor.matmul(db, ones_sb[0:110, 0:110],
                                     exp_t[e], start=True, stop=True)
                    db_t[e] = db
                    rc = p2sb.tile([110, SUB], f32, tag="recip")
                    nc.vector.reciprocal_approx_fast(rc, db_t[e])
                    rc_t[e] = rc
                if 0 <= o < NSUB:          # v-mm (o): PE x4
                    for j, cc in enumerate((2, 3)):
                        nc.tensor.matmul(
                            ob_t[o][:, j, :],
                            vT_sb[:, cc * 128:(cc + 1) * 128],
                            an_t[o], start=False, stop=True,
                            skip_group_check=True)
                    oa = opp.tile([128, 2, SUB], f32, tag="o")
                    for j, cc in enumerate((0, 1)):
                        nc.tensor.matmul(
                            oa[:, j, :],
                            vT_sb[:, cc * 128:(cc + 1) * 128],
                            an_t[o], start=True, stop=True,
                            skip_group_check=True)
                    oa_t[o] = oa
                if 0 <= e < NSUB:          # attn weights (e): GpSimd
                    an = p2sb.tile([110, SUB], f16, tag="attn")
                    with nc.allow_low_precision("softmax in fp16"):
                        nc.gpsimd.tensor_mul(an, exp_t[e], rc_t[e])
                    an_t[e] = an

    nc.compile()
    return nc


_NC_CACHE = None


def _get_nc() -> bacc.Bacc:
    global _NC_CACHE
    if _NC_CACHE is None:
        _NC_CACHE = build_nc()
    return _NC_CACHE


def _prep_in_maps(x, Wq, bq, Wk, bk, Wv, bv):
    A = _build_agg_matrix()
    a_st = np.zeros((128, NSTRIPE, S), np.float32)
    for st, (h0, h1) in enumerate(STRIPE_HB):
        c0, gsz = 24 * h0, 24 * (h1 - h0)
        a_st[:gsz, st, :] = A[c0:c0 + gsz, :]
    Wq64 = Wq.astype(np.float64)
    Wk64 = Wk.astype(np.float64)
    bq64 = bq.astype(np.float64)
    bk64 = bk.astype(np.float64)
    # t = pooled.T (Wk.T bq) + bk.bq == kq.T z + (bk.bq - r.z)
    # where Wq z = bq (z = minimum-norm solution; M z = Wk.T bq exactly)
    z64 = Wq64.T @ np.linalg.solve(Wq64 @ Wq64.T, bq64)
    r64 = Wq64.T @ bk64
    tb = float(bk64 @ bq64 - r64 @ z64)
    shared = {
        "a_st": np.ascontiguousarray(a_st.reshape(128, -1)).astype(F16),
        "m_st": _stage((Wk64.T @ Wq64).astype(np.float32)).astype(F16),
        "wv_st": _stage(np.ascontiguousarray(Wv.T)).astype(F16),
        "z_st": np.ascontiguousarray(
            z64.astype(np.float32).reshape(KT, 128).T).astype(F16),
        "r_st": r64.astype(np.float32).reshape(1, C).astype(F16),
        "bv_st": np.ascontiguousarray(bv.reshape(1, C)).astype(F16),
        "ones_st": np.ones((128, 512), dtype=F16),
        "id_st": np.eye(128, dtype=np.float32).astype(F16),
        "tb_st": np.full((110, 1), tb, dtype=np.float32),
    }
    in_maps = []
    for i in range(NCORES):
        xi_f16 = np.ascontiguousarray(x[i].reshape(C, N)).astype(F16)
        # x: chunk-major [p, (ci: kt, cols)]
        parts = []
        for ci in range(NBIG):
            o, cols = CHUNK_OFF[ci], CHUNK_COLS[ci]
            blk = xi_f16[:, o:o + cols].reshape(KT, 128, cols)
            parts.append(blk.transpose(1, 0, 2).reshape(128, -1))
        x_st = np.ascontiguousarray(np.concatenate(parts, axis=1))
        m = dict(shared)
        m["x_st"] = x_st
        in_maps.append(m)
    return in_maps


def _unstage_out(o: np.ndarray) -> np.ndarray:
    # [128, NSUB*KT*SUB] fp16 -> [C, H, W] fp32
    return np.ascontiguousarray(
        o.reshape(128, NSUB, KT, SUB).transpose(2, 0, 1, 3)
        .reshape(C, N)).astype(np.float32).reshape(C, H, W)


def _install_ntff_hook():
    """The agent image ships no antenv.axon_hooks module, so trace=True
    under axon crashes on import. Recreate the tiny hook-holder module and
    register trn_boot's ctypes NTFF hook so neuron-profile timing works."""
    import sys
    import types
    if "antenv.axon_hooks" in sys.modules:
        return
    mod = types.ModuleType("antenv.axon_hooks")
    holder = {"h": None}
    mod.set_axon_ntff_profile_hook = lambda h: holder.__setitem__("h", h)
    mod.get_axon_ntff_profile_hook = lambda: holder["h"]
    sys.modules["antenv.axon_hooks"] = mod
    try:
        import antenv
        antenv.axon_hooks = mod
    except ImportError:
        pass
    try:
        from trn_agent_boot.trn_boot import _ntff_profile_via_ctypes
        mod.set_axon_ntff_profile_hook(
            _ntff_profile_via_ctypes("/opt/axon/libaxon_pjrt.so"))
    except Exception as e:  # degrade to no profiling
        print(f"ntff hook install failed: {e}")


def _run(trace: bool, **inputs):
    if trace:
        _install_ntff_hook()
        import concourse.bass_utils as bu
        bu.upload_artifacts = lambda tmpdir: tmpdir  # no cloud bucket here
    nc = _get_nc()
    in_maps = _prep_in_maps(
        inputs["x"], inputs["Wq"], inputs["bq"], inputs["Wk"], inputs["bk"],
        inputs["Wv"], inputs["bv"])
    res = run_bass_kernel_spmd(nc, in_maps, core_ids=list(range(NCORES)),
                               trace=trace)
    out = np.stack([
        _unstage_out(np.asarray(res.results[i]["out_st"]))
        for i in range(NCORES)
    ]).astype(np.float32)
    return out, res


def kernel(**inputs) -> np.ndarray:
    out, _ = _run(False, **inputs)
    return out


def kernel_profiled(**inputs):
    out, res = _run(True, **inputs)
    return out, res
